# revision 4
# baseline (speedup 1.0000x reference)
"""Multi-head attention (B=8, N=1024, C=1024, H=16) on 8 TRN2 NeuronCores.

Sharding: data-parallel over batch B=8 -> one batch element per core.
Each core computes, for its batch element:
    qkv = x @ qkv_w.T ; q,k,v split ; per-head softmax(q k^T / sqrt(hd)) v

Device-side layout strategy (all matmuls contract over the SBUF partition dim):
  - host passes xT = x[b].T            [C, N]   (c on partitions)
  - host passes wT = qkv_w.T           [C, 3C]  (c on partitions)
  - qT/kT computed transposed          [d, n]   (head-dim on partitions)
  - v computed in natural layout       [n, dv]  (tokens on partitions), augmented
    with a ones-column so the PV matmul also yields the softmax denominator
  - scores computed transposed S^T=[j,i]; exp fused into the PSUM->SBUF copy
    on the scalar engine; O^T = v_aug.T @ E^T gives [hd+1, i] with row 64 the
    softmax row-sum; normalize with reciprocal * broadcast; DMA out O^T.
  - host transposes the returned outT back to [n, c].

All matmul inputs are float32r (TF32-like reduced-precision fp32 matmul mode,
~1.6e-4 rel err, 4x faster than fp32 matmul on the PE array).
"""

import sys

sys.path.insert(0, "/opt/trn_rl_repo")

import numpy as np

import concourse.bacc as bacc
import concourse.mybir as mybir
import concourse.tile as tile
from concourse.bass_utils import run_bass_kernel_spmd

F32 = mybir.dt.float32
F32R = mybir.dt.float32r
EXP = mybir.ActivationFunctionType.Exp

N = 1024  # tokens
C = 1024  # channels
H = 16    # heads
HD = 64   # head dim
NB = 2    # n blocks of 512
CT = 8    # c tiles of 128
SCALE = HD ** -0.5

# exp via a big SBUF->SBUF activation per (head, i-block) instead of 8 small
# PSUM->SBUF activations (fewer ACT instructions; adds DVE copies)
EXP_FROM_SBUF = False


def build_nc():
    nc = bacc.Bacc(None, target_bir_lowering=False)
    xT_ext = nc.declare_dram_parameter("xT", [C, N], F32R, isOutput=False)
    wT_ext = nc.declare_dram_parameter("wT", [C, 3 * C], F32R, isOutput=False)
    outT_ext = nc.declare_dram_parameter("outT", [C, N], F32, isOutput=True)

    xT3 = xT_ext.rearrange("(co p) n -> p co n", p=128)    # [128, 8, 1024]
    wT3 = wT_ext.rearrange("(co p) d -> p co d", p=128)    # [128, 8, 3072]

    with tile.TileContext(nc) as tc:
        with (
            tc.tile_pool(name="singles", bufs=1) as singles,
            tc.tile_pool(name="psum", bufs=1, space="PSUM") as psum,
        ):
            # ---- load xT (kept resident) ----
            xT_sb = singles.tile([128, CT, N], F32R)
            nc.sync.dma_start(out=xT_sb, in_=xT3)

            # v_aug[p, nt, h, 0:64] = v head h rows; v_aug[p, nt, h, 64] = 1.0
            v_aug = singles.tile([128, CT, H, HD + 1], F32R)
            ones16 = singles.tile([128, H], F32)
            nc.vector.memset(ones16, 1.0)

            # ---- v projection: v[n, dv] = sum_c x[n,c] wv[dv,c] ----
            with tc.tile_pool(name="wvpool", bufs=1) as wvpool:
                wv_sb = wvpool.tile([128, CT, C], F32R)
                nc.sync.dma_start(out=wv_sb, in_=wT3[:, :, 2 * C:3 * C])
                for nt in range(8):
                    for dvb in range(2):
                        ps = psum.tile([128, 512], F32, tag="proj", bufs=2)
                        for ct in range(CT):
                            nc.tensor.matmul(
                                ps,
                                xT_sb[:, ct, nt * 128:(nt + 1) * 128],
                                wv_sb[:, ct, dvb * 512:(dvb + 1) * 512],
                                start=(ct == 0),
                                stop=(ct == CT - 1),
                            )
                        h0 = dvb * 8
                        nc.vector.tensor_copy(
                            v_aug[:, nt, h0:h0 + 8, 0:HD],
                            ps.rearrange("p (h e) -> p h e", h=8),
                        )
                    nc.vector.tensor_copy(v_aug[:, nt, :, HD], ones16)

            # ---- per head-pair: qk projection, scores, softmax, PV ----
            with (
                tc.tile_pool(name="wqkpool", bufs=2) as wqkpool,
                tc.tile_pool(name="qkpool", bufs=2) as qkpool,
                tc.tile_pool(name="epool", bufs=2) as epool,
                tc.tile_pool(name="opool", bufs=3) as opool,
            ):
                for t in range(8):  # head pairs (2t, 2t+1)
                    wqk = wqkpool.tile([128, CT, 256], F32R, tag="wqk")
                    nc.sync.dma_start(
                        out=wqk[:, :, 0:128], in_=wT3[:, :, t * 128:(t + 1) * 128]
                    )
                    nc.sync.dma_start(
                        out=wqk[:, :, 128:256],
                        in_=wT3[:, :, C + t * 128:C + (t + 1) * 128],
                    )

                    qT = qkpool.tile([128, N], F32R, tag="qT")
                    kT = qkpool.tile([128, N], F32R, tag="kT")
                    for dst, off in ((qT, 0), (kT, 128)):
                        for nb in range(NB):
                            ps = psum.tile([128, 512], F32, tag="proj", bufs=2)
                            for ct in range(CT):
                                nc.tensor.matmul(
                                    ps,
                                    wqk[:, ct, off:off + 128],
                                    xT_sb[:, ct, nb * 512:(nb + 1) * 512],
                                    start=(ct == 0),
                                    stop=(ct == CT - 1),
                                )
                            nc.vector.tensor_copy(
                                dst[:, nb * 512:(nb + 1) * 512], ps
                            )

                    for ib in range(NB):  # query blocks of 512
                        ibs = slice(ib * 512, (ib + 1) * 512)
                        # scores^T + exp for both heads of the pair, interleaved
                        # so the two K=64 matmuls overlap on disjoint row groups
                        Es = [
                            epool.tile([128, CT, 512], F32R, tag=f"E{hh}",
                                       name=f"E{hh}")
                            for hh in range(2)
                        ]
                        for jt in range(8):
                            jts = slice(jt * 128, (jt + 1) * 128)
                            sps = [None, None]
                            for hh in range(2):
                                p0 = hh * 64
                                sps[hh] = psum.tile([128, 512], F32, tag="s",
                                                    bufs=4, name=f"sps{hh}")
                                nc.tensor.matmul(
                                    sps[hh],
                                    kT[p0:p0 + 64, jts],
                                    qT[p0:p0 + 64, ibs],
                                    start=True,
                                    stop=True,
                                )
                            for hh in range(2):
                                if EXP_FROM_SBUF:
                                    nc.vector.tensor_copy(Es[hh][:, jt, :], sps[hh])
                                else:
                                    nc.scalar.activation(
                                        out=Es[hh][:, jt, :], in_=sps[hh],
                                        func=EXP, scale=SCALE,
                                    )
                        if EXP_FROM_SBUF:
                            for hh in range(2):
                                nc.scalar.activation(
                                    out=Es[hh], in_=Es[hh].bitcast(F32),
                                    func=EXP, scale=SCALE,
                                )
                        for hh in range(2):
                            h = 2 * t + hh
                            pv = psum.tile([HD + 1, 512], F32, tag="pv", bufs=2)
                            for jt in range(8):
                                nc.tensor.matmul(
                                    pv,
                                    v_aug[:, jt, h, :],
                                    Es[hh][:, jt, :],
                                    start=(jt == 0),
                                    stop=(jt == 7),
                                )
                            # normalize: rows 0:64 divided by row 64 (the sum)
                            srec = opool.tile([1, 512], F32, tag="srec")
                            nc.vector.reciprocal(srec, pv[HD:HD + 1, :])
                            bcast = opool.tile([HD, 512], F32, tag="bcast")
                            nc.gpsimd.partition_broadcast(bcast, srec)
                            osb = opool.tile([HD, 512], F32, tag="osb")
                            nc.vector.tensor_mul(osb, pv[0:HD, :], bcast)
                            nc.sync.dma_start(
                                out=outT_ext[h * HD:(h + 1) * HD, ibs], in_=osb
                            )
    nc.compile()
    return nc


_NC_CACHE = {}


def _get_nc():
    if "nc" not in _NC_CACHE:
        _NC_CACHE["nc"] = build_nc()
    return _NC_CACHE["nc"]


def kernel(x: np.ndarray, qkv_w: np.ndarray, _trace: bool = False):
    B = x.shape[0]
    assert x.shape == (B, N, C) and qkv_w.shape == (3 * C, C)
    wT = np.ascontiguousarray(qkv_w.T.astype(np.float32))
    in_maps = [
        {"xT": np.ascontiguousarray(x[b].T.astype(np.float32)), "wT": wT}
        for b in range(B)
    ]
    nc = _get_nc()
    res = run_bass_kernel_spmd(
        nc, in_maps, core_ids=list(range(8)), trace=_trace
    )
    out = np.stack([res.results[b]["outT"].T for b in range(B)])
    if _trace:
        return out, res
    return out


# revision 7
# speedup vs baseline: 1.2982x; 1.2982x over previous
"""Multi-head attention (B=8, N=1024, C=1024, H=16) on 8 TRN2 NeuronCores.

Sharding: data-parallel over batch B=8 -> one batch element per core.
Each core computes, for its batch element:
    qkv = x @ qkv_w.T ; q,k,v split ; per-head softmax(q k^T / sqrt(hd)) v

Device-side layout strategy (all matmuls contract over the SBUF partition dim):
  - host passes xT = x[b].T            [C, N]   (c on partitions)
  - host passes wT = qkv_w.T           [C, 3C]  (c on partitions)
  - qT/kT computed transposed          [d, n]   (head-dim on partitions)
  - v computed in natural layout       [n, dv]  (tokens on partitions), augmented
    with a ones-column so the PV matmul also yields the softmax denominator
  - scores computed transposed S^T=[j,i]; exp fused into the PSUM->SBUF copy
    on the scalar engine; O^T = v_aug.T @ E^T gives [hd+1, i] with row 64 the
    softmax row-sum; normalize with ACT reciprocal + gpsimd partition
    broadcast + DVE multiply; DMA out O^T.
  - host transposes the returned outT back to [n, c].

All matmul inputs are float32r (TF32-like reduced-precision fp32 matmul mode,
~1.6e-4 rel err, 4x faster than fp32 matmul on the PE array).
"""

import sys

sys.path.insert(0, "/opt/trn_rl_repo")

import numpy as np

import concourse.bacc as bacc
import concourse.mybir as mybir
import concourse.tile as tile
from concourse.bass_utils import run_bass_kernel_spmd

F32 = mybir.dt.float32
F32R = mybir.dt.float32r
EXP = mybir.ActivationFunctionType.Exp
LN = mybir.ActivationFunctionType.Ln

N = 1024  # tokens
C = 1024  # channels
H = 16    # heads
HD = 64   # head dim
NB = 2    # n blocks of 512
CT = 8    # c tiles of 128
SCALE = HD ** -0.5


def build_nc():
    nc = bacc.Bacc(None, target_bir_lowering=False)
    xT_ext = nc.declare_dram_parameter("xT", [C, N], F32R, isOutput=False)
    wT_ext = nc.declare_dram_parameter("wT", [C, 3 * C], F32R, isOutput=False)
    outT_ext = nc.declare_dram_parameter("outT", [C, N], F32, isOutput=True)

    xT3 = xT_ext.rearrange("(co p) n -> p co n", p=128)    # [128, 8, 1024]
    wT3 = wT_ext.rearrange("(co p) d -> p co d", p=128)    # [128, 8, 3072]

    with tile.TileContext(nc) as tc:
        with (
            tc.tile_pool(name="singles", bufs=1) as singles,
            tc.tile_pool(name="psum", bufs=1, space="PSUM") as psum,
        ):
            # ---- load xT (kept resident) ----
            xT_sb = singles.tile([128, CT, N], F32R)
            nc.sync.dma_start(out=xT_sb, in_=xT3)

            # v_aug[p, nt, h, 0:64] = v head h rows; v_aug[p, nt, h, 64] = 1.0
            v_aug = singles.tile([128, CT, H, HD + 1], F32R)
            ones16 = singles.tile([128, H], F32)
            nc.vector.memset(ones16, 1.0)

            # ---- v projection: v[n, dv] = sum_c x[n,c] wv[dv,c] ----
            # stationary (lhsT) = xT tile, reused across the two dv blocks
            with tc.tile_pool(name="wvpool", bufs=1) as wvpool:
                wv_sb = wvpool.tile([128, CT, C], F32R)
                nc.sync.dma_start(out=wv_sb, in_=wT3[:, :, 2 * C:3 * C])
                for nt in range(8):
                    pss = [
                        psum.tile([128, 512], F32, tag="proj", bufs=2,
                                  name=f"vps{dvb}")
                        for dvb in range(2)
                    ]
                    for ct in range(CT):
                        for dvb in range(2):
                            nc.tensor.matmul(
                                pss[dvb],
                                xT_sb[:, ct, nt * 128:(nt + 1) * 128],
                                wv_sb[:, ct, dvb * 512:(dvb + 1) * 512],
                                start=(ct == 0),
                                stop=(ct == CT - 1),
                            )
                    for dvb in range(2):
                        h0 = dvb * 8
                        nc.vector.tensor_copy(
                            v_aug[:, nt, h0:h0 + 8, 0:HD],
                            pss[dvb].rearrange("p (h e) -> p h e", h=8),
                        )
                    nc.vector.tensor_copy(v_aug[:, nt, :, HD], ones16)

            # ---- per head-pair: qk projection, scores, softmax, PV ----
            with (
                tc.tile_pool(name="wqkpool", bufs=2) as wqkpool,
                tc.tile_pool(name="qkpool", bufs=2) as qkpool,
                tc.tile_pool(name="epool", bufs=2) as epool,
                tc.tile_pool(name="opool", bufs=3) as opool,
            ):
                for t in range(8):  # head pairs (2t, 2t+1)
                    wqk = wqkpool.tile([128, CT, 256], F32R, tag="wqk")
                    nc.sync.dma_start(
                        out=wqk[:, :, 0:128], in_=wT3[:, :, t * 128:(t + 1) * 128]
                    )
                    nc.sync.dma_start(
                        out=wqk[:, :, 128:256],
                        in_=wT3[:, :, C + t * 128:C + (t + 1) * 128],
                    )

                    # stationary (lhsT) = w tile, reused across the two n blocks
                    qT = qkpool.tile([128, N], F32R, tag="qT")
                    kT = qkpool.tile([128, N], F32R, tag="kT")
                    for dst, off in ((qT, 0), (kT, 128)):
                        pss = [
                            psum.tile([128, 512], F32, tag="proj", bufs=2,
                                      name=f"qkps{nb}")
                            for nb in range(NB)
                        ]
                        for ct in range(CT):
                            for nb in range(NB):
                                nc.tensor.matmul(
                                    pss[nb],
                                    wqk[:, ct, off:off + 128],
                                    xT_sb[:, ct, nb * 512:(nb + 1) * 512],
                                    start=(ct == 0),
                                    stop=(ct == CT - 1),
                                )
                        for nb in range(NB):
                            nc.vector.tensor_copy(
                                dst[:, nb * 512:(nb + 1) * 512], pss[nb]
                            )

                    for ib in range(NB):  # query blocks of 512
                        ibs = slice(ib * 512, (ib + 1) * 512)
                        # scores^T + exp for both heads of the pair, interleaved
                        # so the two K=64 matmuls overlap on disjoint row groups.
                        # S psum tiles span 2 banks (2 j-tiles) so each exp
                        # activation covers 1024 columns.
                        Es = [
                            epool.tile([128, CT, 512], F32R, tag=f"E{hh}",
                                       name=f"E{hh}")
                            for hh in range(2)
                        ]
                        for jp in range(4):  # pairs of j-tiles
                            sps = [None, None]
                            for hh in range(2):
                                p0 = hh * 64
                                sps[hh] = psum.tile([128, 1024], F32, tag="s",
                                                    bufs=2, name=f"sps{hh}")
                                for jh in range(2):
                                    jt = 2 * jp + jh
                                    nc.tensor.matmul(
                                        sps[hh][:, jh * 512:(jh + 1) * 512],
                                        kT[p0:p0 + 64, jt * 128:(jt + 1) * 128],
                                        qT[p0:p0 + 64, ibs],
                                        start=True,
                                        stop=True,
                                    )
                            for hh in range(2):
                                nc.scalar.activation(
                                    out=Es[hh][:, 2 * jp:2 * jp + 2, :],
                                    in_=sps[hh],
                                    func=EXP, scale=SCALE,
                                )
                        for hh in range(2):
                            h = 2 * t + hh
                            pv = psum.tile([HD + 1, 512], F32, tag="pv", bufs=2)
                            for jt in range(8):
                                nc.tensor.matmul(
                                    pv,
                                    v_aug[:, jt, h, :],
                                    Es[hh][:, jt, :],
                                    start=(jt == 0),
                                    stop=(jt == 7),
                                )
                            # normalize: rows 0:64 divided by row 64 (the sum)
                            # 1/x as exp(-ln(x)) (ACT Reciprocal is blocked)
                            slog = opool.tile([1, 512], F32, tag="slog")
                            nc.scalar.activation(
                                out=slog, in_=pv[HD:HD + 1, :], func=LN
                            )
                            srec = opool.tile([1, 512], F32, tag="srec")
                            nc.scalar.activation(
                                out=srec, in_=slog, func=EXP, scale=-1.0
                            )
                            bcast = opool.tile([HD, 512], F32, tag="bcast")
                            nc.gpsimd.partition_broadcast(bcast, srec)
                            osb = opool.tile([HD, 512], F32, tag="osb")
                            nc.vector.tensor_mul(osb, pv[0:HD, :], bcast)
                            nc.sync.dma_start(
                                out=outT_ext[h * HD:(h + 1) * HD, ibs], in_=osb
                            )
    nc.compile()
    return nc


_NC_CACHE = {}


def _get_nc():
    if "nc" not in _NC_CACHE:
        _NC_CACHE["nc"] = build_nc()
    return _NC_CACHE["nc"]


def kernel(x: np.ndarray, qkv_w: np.ndarray, _trace: bool = False):
    B = x.shape[0]
    assert x.shape == (B, N, C) and qkv_w.shape == (3 * C, C)
    wT = np.ascontiguousarray(qkv_w.T.astype(np.float32))
    in_maps = [
        {"xT": np.ascontiguousarray(x[b].T.astype(np.float32)), "wT": wT}
        for b in range(B)
    ]
    nc = _get_nc()
    res = run_bass_kernel_spmd(
        nc, in_maps, core_ids=list(range(8)), trace=_trace
    )
    out = np.stack([res.results[b]["outT"].T for b in range(B)])
    if _trace:
        return out, res
    return out


# revision 8
# speedup vs baseline: 1.4922x; 1.1494x over previous
"""Multi-head attention (B=8, N=1024, C=1024, H=16) on 8 TRN2 NeuronCores.

Sharding: data-parallel over batch B=8 -> one batch element per core.
Each core computes, for its batch element:
    qkv = x @ qkv_w.T ; q,k,v split ; per-head softmax(q k^T / sqrt(hd)) v

Device-side layout strategy (all matmuls contract over the SBUF partition dim):
  - host passes xT = x[b].T (bf16)     [C, N]   (c on partitions)
  - host passes wT = qkv_w.T (bf16)    [C, 3C]  (c on partitions)
  - qT/kT computed transposed          [d, n]   (head-dim on partitions)
  - v computed in natural layout       [n, dv]  (tokens on partitions), augmented
    with a ones-column so the PV matmul also yields the softmax denominator
  - scores computed transposed S^T=[j,i]; exp fused into the PSUM->SBUF copy
    on the scalar engine (bf16 out); O^T = v_aug.T @ E^T gives [hd+1, i] with
    row 64 the softmax row-sum; reciprocal done at [128,4] layout via a DRAM
    bounce (keeps the DVE reciprocal on all lanes), broadcast back via a
    stride-0 DRAM read; DMA out O^T.
  - host transposes the returned outT back to [n, c].

All matmul inputs are bf16 (PSUM accumulation in fp32); measured end-to-end
relative error ~5e-3 vs the fp32 reference (gate is 2e-2).
"""

import sys

sys.path.insert(0, "/opt/trn_rl_repo")

import ml_dtypes
import numpy as np

import concourse.bacc as bacc
import concourse.mybir as mybir
import concourse.tile as tile
from concourse.bass_utils import run_bass_kernel_spmd

F32 = mybir.dt.float32
BF16 = mybir.dt.bfloat16
EXP = mybir.ActivationFunctionType.Exp

N = 1024  # tokens
C = 1024  # channels
H = 16    # heads
HD = 64   # head dim
NB = 2    # n blocks of 512
CT = 8    # c tiles of 128
SCALE = HD ** -0.5


def build_nc():
    nc = bacc.Bacc(None, target_bir_lowering=False)
    xT_ext = nc.declare_dram_parameter("xT", [C, N], BF16, isOutput=False)
    wT_ext = nc.declare_dram_parameter("wT", [C, 3 * C], BF16, isOutput=False)
    outT_ext = nc.declare_dram_parameter("outT", [C, N], F32, isOutput=True)

    xT3 = xT_ext.rearrange("(co p) n -> p co n", p=128)    # [128, 8, 1024]
    wT3 = wT_ext.rearrange("(co p) d -> p co d", p=128)    # [128, 8, 3072]

    with tile.TileContext(nc) as tc:
        with (
            tc.tile_pool(name="singles", bufs=1) as singles,
            tc.tile_pool(name="psum", bufs=1, space="PSUM") as psum,
            tc.tile_pool(name="drp", bufs=4, space="DRAM") as drp,
        ):
            # ---- load xT (kept resident), split per c-tile for early start ----
            xT_sb = singles.tile([128, CT, N], BF16)
            for ct in range(CT):
                nc.sync.dma_start(out=xT_sb[:, ct, :], in_=xT3[:, ct, :])

            # v_aug[p, nt, h, 0:64] = v head h rows; v_aug[p, nt, h, 64] = 1.0
            v_aug = singles.tile([128, CT, H, HD + 1], BF16)
            ones16 = singles.tile([128, H], F32)
            nc.vector.memset(ones16, 1.0)

            # ---- v projection: v[n, dv] = sum_c x[n,c] wv[dv,c] ----
            # stationary (lhsT) = xT tile, reused across the two dv blocks
            with tc.tile_pool(name="wvpool", bufs=1) as wvpool:
                wv_sb = wvpool.tile([128, CT, C], BF16)
                for ct in range(CT):
                    nc.sync.dma_start(
                        out=wv_sb[:, ct, :], in_=wT3[:, ct, 2 * C:3 * C]
                    )
                for nt in range(8):
                    pss = [
                        psum.tile([128, 512], F32, tag="proj", bufs=2,
                                  name=f"vps{dvb}")
                        for dvb in range(2)
                    ]
                    for ct in range(CT):
                        for dvb in range(2):
                            nc.tensor.matmul(
                                pss[dvb],
                                xT_sb[:, ct, nt * 128:(nt + 1) * 128],
                                wv_sb[:, ct, dvb * 512:(dvb + 1) * 512],
                                start=(ct == 0),
                                stop=(ct == CT - 1),
                            )
                    for dvb in range(2):
                        h0 = dvb * 8
                        nc.vector.tensor_copy(
                            v_aug[:, nt, h0:h0 + 8, 0:HD],
                            pss[dvb].rearrange("p (h e) -> p h e", h=8),
                        )
                    nc.vector.tensor_copy(v_aug[:, nt, :, HD], ones16)

            # ---- per head-pair: qk projection, scores, softmax, PV ----
            with (
                tc.tile_pool(name="wqkpool", bufs=2) as wqkpool,
                tc.tile_pool(name="qkpool", bufs=2) as qkpool,
                tc.tile_pool(name="epool", bufs=2) as epool,
                tc.tile_pool(name="opool", bufs=3) as opool,
            ):
                for t in range(8):  # head pairs (2t, 2t+1)
                    wqk = wqkpool.tile([128, CT, 256], BF16, tag="wqk")
                    nc.sync.dma_start(
                        out=wqk[:, :, 0:128], in_=wT3[:, :, t * 128:(t + 1) * 128]
                    )
                    nc.sync.dma_start(
                        out=wqk[:, :, 128:256],
                        in_=wT3[:, :, C + t * 128:C + (t + 1) * 128],
                    )

                    # stationary (lhsT) = w tile, reused across the two n blocks
                    qT = qkpool.tile([128, N], BF16, tag="qT")
                    kT = qkpool.tile([128, N], BF16, tag="kT")
                    for dst, off in ((qT, 0), (kT, 128)):
                        pss = [
                            psum.tile([128, 512], F32, tag="proj", bufs=2,
                                      name=f"qkps{nb}")
                            for nb in range(NB)
                        ]
                        for ct in range(CT):
                            for nb in range(NB):
                                nc.tensor.matmul(
                                    pss[nb],
                                    wqk[:, ct, off:off + 128],
                                    xT_sb[:, ct, nb * 512:(nb + 1) * 512],
                                    start=(ct == 0),
                                    stop=(ct == CT - 1),
                                )
                        for nb in range(NB):
                            nc.vector.tensor_copy(
                                dst[:, nb * 512:(nb + 1) * 512], pss[nb]
                            )

                    for ib in range(NB):  # query blocks of 512
                        ibs = slice(ib * 512, (ib + 1) * 512)
                        # scores^T + exp for both heads of the pair, interleaved
                        # so the two K=64 matmuls overlap on disjoint row groups.
                        # S psum tiles span 2 banks (2 j-tiles) so each exp
                        # activation covers 1024 columns.
                        Es = [
                            epool.tile([128, CT, 512], BF16, tag=f"E{hh}",
                                       name=f"E{hh}")
                            for hh in range(2)
                        ]
                        for jp in range(4):  # pairs of j-tiles
                            sps = [None, None]
                            for hh in range(2):
                                p0 = hh * 64
                                sps[hh] = psum.tile([128, 1024], F32, tag="s",
                                                    bufs=2, name=f"sps{hh}")
                                for jh in range(2):
                                    jt = 2 * jp + jh
                                    nc.tensor.matmul(
                                        sps[hh][:, jh * 512:(jh + 1) * 512],
                                        kT[p0:p0 + 64, jt * 128:(jt + 1) * 128],
                                        qT[p0:p0 + 64, ibs],
                                        start=True,
                                        stop=True,
                                    )
                            for hh in range(2):
                                nc.scalar.activation(
                                    out=Es[hh][:, 2 * jp:2 * jp + 2, :],
                                    in_=sps[hh],
                                    func=EXP, scale=SCALE,
                                )
                        for hh in range(2):
                            h = 2 * t + hh
                            pv = psum.tile([HD + 1, 512], F32, tag="pv", bufs=2)
                            for jt in range(8):
                                nc.tensor.matmul(
                                    pv,
                                    v_aug[:, jt, h, :],
                                    Es[hh][:, jt, :],
                                    start=(jt == 0),
                                    stop=(jt == 7),
                                )
                            # normalize rows 0:64 by row 64 (the softmax sum):
                            # sums -> DRAM -> [128,4] so the reciprocal runs on
                            # all DVE lanes -> DRAM -> stride-0 partition
                            # broadcast read -> multiply
                            srow = opool.tile([1, 512], F32, tag="srow")
                            nc.vector.tensor_copy(srow, pv[HD:HD + 1, :])
                            dr = drp.tile([1, 512], F32, tag="dr")
                            nc.sync.dma_start(out=dr, in_=srow)
                            r4 = opool.tile([128, 4], F32, tag="r4")
                            nc.sync.dma_start(
                                out=r4, in_=dr.rearrange("o (p f) -> (o p) f", p=128)
                            )
                            r4b = opool.tile([128, 4], F32, tag="r4b")
                            nc.vector.reciprocal(r4b, r4)
                            dr2 = drp.tile([1, 512], F32, tag="dr2")
                            nc.sync.dma_start(
                                out=dr2.rearrange("o (p f) -> (o p) f", p=128),
                                in_=r4b,
                            )
                            bcast = opool.tile([HD, 512], F32, tag="bcast")
                            nc.sync.dma_start(
                                out=bcast, in_=dr2.partition_broadcast(HD).squeeze(1)
                            )
                            osb = opool.tile([HD, 512], F32, tag="osb")
                            nc.vector.tensor_mul(osb, pv[0:HD, :], bcast)
                            nc.sync.dma_start(
                                out=outT_ext[h * HD:(h + 1) * HD, ibs], in_=osb
                            )
    nc.compile()
    return nc


_NC_CACHE = {}


def _get_nc():
    if "nc" not in _NC_CACHE:
        _NC_CACHE["nc"] = build_nc()
    return _NC_CACHE["nc"]


def kernel(x: np.ndarray, qkv_w: np.ndarray, _trace: bool = False):
    B = x.shape[0]
    assert x.shape == (B, N, C) and qkv_w.shape == (3 * C, C)
    bf = ml_dtypes.bfloat16
    wT = np.ascontiguousarray(qkv_w.T).astype(bf)
    in_maps = [
        {"xT": np.ascontiguousarray(x[b].T).astype(bf), "wT": wT}
        for b in range(B)
    ]
    nc = _get_nc()
    res = run_bass_kernel_spmd(
        nc, in_maps, core_ids=list(range(8)), trace=_trace
    )
    out = np.stack([res.results[b]["outT"].T for b in range(B)])
    if _trace:
        return out, res
    return out
